# revision 24
# baseline (speedup 1.0000x reference)
"""EvolveGCN forward on 8 Trainium2 NeuronCores.

Strategy (SPMD, one program on 8 cores):
  - Nodes sharded across cores (12500/core, padded to 12544 = 98 blocks of 128).
  - Edges partitioned by destination row; scatter stays core-local via
    one-hot matmul accumulation in PSUM (dest block = 128 psum partitions).
  - Source features gathered per-edge from a replicated table in core-local
    HBM (AllGather'd between layers) with dma_gather (int16 idx, table split
    in 4 quarters of 25088 rows to fit int16 range).
  - GCN norm folded as: t = relu(h@W+b) * dinv[node]  (source fold, on store)
    and h_block = psum * dinv[dest] (dest fold, on PSUM evacuation).
  - Dense transforms run feature-major (weights stationary on TensorE);
    PE transposes convert to node-major for edge gathers.

Host side: edge bucketing by (core, dest block, source quarter), degree/
dinv computation, x transpose, and final output assembly.
"""

import os
import numpy as np

# ---- problem constants (hardcoded per contract) ----
N = 100000
E = 1600000
F_IN = 128
H = 64
NCORES = 8
SHARD = 12500          # real nodes per core
SHARD_PAD = 12544      # = 98 * 128
B = SHARD_PAD // 128   # 98 dest blocks per core
NQ = 4                 # table quarters (int16 index range)
QROWS = SHARD_PAD * NCORES // NQ   # 25088 rows per quarter
NPAD = SHARD_PAD * NCORES          # 100352
GMERGE = 4                         # dest blocks per merged gather call

_BUILD_CACHE = {}
LAST_RESULTS = None     # BassKernelResults of the most recent run (for test harness)


def _preprocess_edges(edge_index):
    """Bucket undirected edges by (core, dest block, source quarter).

    Per-(block, quarter) exact slot counts: JQ_bq = max over cores of
    ceil(count/128) (shared across cores so the SPMD program has uniform
    loop bounds).  Layouts are concatenated along the free axis in (b, q)
    order:
      colidx_i16: [NC, 128, 8*TOTJ] int16 — dma_gather index tiles
                  (16-partition wrap, replicated through 128 partitions)
      lrow_f32:   [NC, 128, TOTJ] float32 — local dest row per slot,
                  -1 for padding slots
      dinv:       [N] float32 — deg^-0.5 (0 for isolated nodes)
      jq_tab:     [B, NQ] int — slots/(128) per group
    """
    e0 = np.asarray(edge_index[0], dtype=np.int64)
    e1 = np.asarray(edge_index[1], dtype=np.int64)
    row = np.concatenate([e0, e1]).astype(np.int32)
    col = np.concatenate([e1, e0]).astype(np.int32)

    deg = np.bincount(row, minlength=N).astype(np.float32)
    with np.errstate(divide="ignore"):
        dinv = np.where(deg > 0, deg.astype(np.float32) ** -0.5, 0.0).astype(np.float32)

    core = row // SHARD
    r_local = row - core * SHARD
    block = r_local >> 7          # // 128
    lr = r_local & 127            # % 128
    # chunked-AllGather table layout: quarter q holds local rows
    # [q*CHUNK, (q+1)*CHUNK) of every rank, rank-major within the quarter
    CHUNK = SHARD_PAD // NQ       # 3136
    c_rank = col // SHARD
    c_loc = col % SHARD
    quarter = c_loc // CHUNK
    c_local = c_rank * CHUNK + c_loc % CHUNK

    # group id and stable ordering
    gid = ((core * B + block) * NQ + quarter).astype(np.int64)
    order = np.argsort(gid, kind="stable")
    gid_s = gid[order]
    lr_s = lr[order].astype(np.float32)
    cl_s = c_local[order].astype(np.int16)

    ngroups = NCORES * B * NQ
    counts = np.bincount(gid_s, minlength=ngroups)
    # per-(b, q) slot count, maxed over cores for SPMD-uniform loop bounds
    jq_tab = np.ceil(counts.reshape(NCORES, B, NQ).max(axis=0) / 128
                     ).astype(np.int64)                      # [B, NQ]
    np.maximum(jq_tab, 1, out=jq_tab)
    TOTJ = int(jq_tab.sum())

    starts = np.zeros(ngroups, dtype=np.int64)
    starts[1:] = np.cumsum(counts)[:-1]
    # offset of each edge within its group
    off = np.arange(len(gid_s), dtype=np.int64) - starts[gid_s]

    # per-group slot base within the concatenated (bg, q, b-in-group) layout
    # (merged gather calls: blocks grouped GMERGE at a time, same quarter)
    order = []
    for b0 in range(0, B, GMERGE):
        for q in range(NQ):
            for b in range(b0, min(b0 + GMERGE, B)):
                order.append(b * NQ + q)
    slot_base = np.zeros(B * NQ, dtype=np.int64)
    acc = 0
    for g in order:
        slot_base[g] = acc
        acc += int(jq_tab.reshape(-1)[g]) * 128

    # flat slot arrays per core; pad idx -> 0 (gathers row 0, zeroed by the
    # lrow=-1 one-hot; trailing -1 indices would skip descriptor generation
    # but crash the NRT worker, so padding stays at idx 0)
    colidx_flat = np.zeros((NCORES, TOTJ * 128), dtype=np.int16)
    lrow_flat = np.full((NCORES, TOTJ * 128), -1.0, dtype=np.float32)
    core_s = gid_s // (B * NQ)
    grp_s = gid_s % (B * NQ)
    pos = slot_base[grp_s] + off
    colidx_flat[core_s, pos] = cl_s
    lrow_flat[core_s, pos] = lr_s

    # per-group dma_gather index layout: slot i -> [16*rep + i%16, i//16];
    # lrow layout: slot i = j*128 + p -> [p, j]
    colidx_i16 = np.zeros((NCORES, 128, 8 * TOTJ), dtype=np.int16)
    lrow_f32 = np.full((NCORES, 128, TOTJ), -1.0, dtype=np.float32)
    jqs = jq_tab.reshape(-1)
    for g in order:
        jq = int(jqs[g])
        s0 = slot_base[g]
        o = s0 // 128
        ci = colidx_flat[:, s0:s0 + jq * 128].reshape(NCORES, 8 * jq, 16)
        ci = ci.transpose(0, 2, 1)                        # [NC, 16, 8jq]
        colidx_i16[:, :, 8 * o:8 * (o + jq)] = np.tile(ci, (1, 8, 1))
        lw = lrow_flat[:, s0:s0 + jq * 128].reshape(NCORES, jq, 128)
        lrow_f32[:, :, o:o + jq] = lw.transpose(0, 2, 1)

    return colidx_i16, lrow_f32, dinv, jq_tab


def _build(jq_tab, phases=3, no_ag=False):
    """Build + compile the SPMD Bass program. Returns (nc, meta).

    phases: 1 = transform + AG1 only, 2 = + layer-1 propagate + AG2,
            3 = full kernel. no_ag replaces AllGathers with a local copy
    (numerically wrong; for timing breakdown only).
    """
    import concourse.bass as bass
    import concourse.mybir as mybir
    import concourse.tile as tile
    from concourse import bacc
    from concourse.masks import make_identity

    fp32 = mybir.dt.float32
    i16 = mybir.dt.int16

    jqs = [[int(jq_tab[b][q]) for q in range(NQ)] for b in range(B)]
    TOTJ = sum(sum(r) for r in jqs)
    # free-axis offset (in J units) of each (b, q) group, in the merged
    # (bg, q, b-in-group) order used by _preprocess_edges
    joff = [[0] * NQ for _ in range(B)]
    acc_j = 0
    for b0 in range(0, B, GMERGE):
        for q in range(NQ):
            for b in range(b0, min(b0 + GMERGE, B)):
                joff[b][q] = acc_j
                acc_j += jqs[b][q]
    # max summed J per merged call (gather buffer size)
    JGMAX = max(
        sum(jqs[b][q] for b in range(b0, min(b0 + GMERGE, B)))
        for b0 in range(0, B, GMERGE) for q in range(NQ)
    )

    nc = bacc.Bacc("TRN2", target_bir_lowering=False, debug=False, num_devices=NCORES)

    # ---- I/O tensors (per-core data via in_maps) ----
    xT = nc.dram_tensor("xT", [F_IN, SHARD_PAD], fp32, kind="ExternalInput").ap()
    W_in = nc.dram_tensor("W_in", [F_IN, H], fp32, kind="ExternalInput").ap()
    W1 = nc.dram_tensor("W1", [H, H], fp32, kind="ExternalInput").ap()
    W2 = nc.dram_tensor("W2", [H, H], fp32, kind="ExternalInput").ap()
    W_out = nc.dram_tensor("W_out", [H, H], fp32, kind="ExternalInput").ap()
    b_in = nc.dram_tensor("b_in", [H, 1], fp32, kind="ExternalInput").ap()
    b1 = nc.dram_tensor("b1", [H, 1], fp32, kind="ExternalInput").ap()
    b2 = nc.dram_tensor("b2", [H, 1], fp32, kind="ExternalInput").ap()
    b_out = nc.dram_tensor("b_out", [H, 1], fp32, kind="ExternalInput").ap()
    dinv_cols = nc.dram_tensor("dinv_cols", [128, B], fp32, kind="ExternalInput").ap()
    iota_in = nc.dram_tensor("iota_in", [128, 128], fp32, kind="ExternalInput").ap()
    colidx = nc.dram_tensor("colidx", [128, 8 * TOTJ], i16, kind="ExternalInput").ap()
    lrow = nc.dram_tensor("lrow", [128, TOTJ], fp32, kind="ExternalInput").ap()
    outT = nc.dram_tensor("outT", [H, SHARD_PAD], fp32, kind="ExternalOutput").ap()

    # internal DRAM
    t_own = nc.dram_tensor("t_own", [SHARD_PAD, H], fp32)
    t_full1 = nc.dram_tensor("t_full1", [NPAD, H], fp32, addr_space="Shared")
    t_own2 = nc.dram_tensor("t_own2", [SHARD_PAD, H], fp32)
    t_full2 = nc.dram_tensor("t_full2", [NPAD, H], fp32, addr_space="Shared")

    groups = [list(range(NCORES))]

    with tile.TileContext(nc) as tc:
        with tc.tile_pool(name="const", bufs=1) as cpool:
            # constants resident for the whole kernel
            iota_t = cpool.tile([128, 128], fp32)
            nc.sync.dma_start(iota_t[:], iota_in[:])
            ident = cpool.tile([128, 128], fp32)
            make_identity(nc, ident[:])
            w_in_t = cpool.tile([F_IN, H], fp32)
            nc.sync.dma_start(w_in_t[:], W_in[:])
            w1_t = cpool.tile([H, H], fp32)
            nc.sync.dma_start(w1_t[:], W1[:])
            w2_t = cpool.tile([H, H], fp32)
            nc.sync.dma_start(w2_t[:], W2[:])
            w_out_t = cpool.tile([H, H], fp32)
            nc.sync.dma_start(w_out_t[:], W_out[:])
            bin_t = cpool.tile([H, 1], fp32)
            nc.sync.dma_start(bin_t[:], b_in[:])
            b1_t = cpool.tile([H, 1], fp32)
            nc.sync.dma_start(b1_t[:], b1[:])
            b2_t = cpool.tile([H, 1], fp32)
            nc.sync.dma_start(b2_t[:], b2[:])
            bout_t = cpool.tile([H, 1], fp32)
            nc.sync.dma_start(bout_t[:], b_out[:])
            dinv_t = cpool.tile([128, B], fp32)
            nc.sync.dma_start(dinv_t[:], dinv_cols[:])

            # ---- Phase A: t1 = relu((x@W_in + b_in)@W1 + b1) * dinv ----
            T = 512
            with tc.tile_pool(name="xf", bufs=3) as xf, \
                 tc.tile_pool(name="xfp", bufs=2, space="PSUM") as xfp:
                ag1_done = [False] * NQ
                CHK = SHARD_PAD // NQ

                def emit_ag1(rows_done):
                    if no_ag:
                        return
                    for c in range(NQ):
                        if not ag1_done[c] and rows_done >= (c + 1) * CHK + 512:
                            ag1_done[c] = True
                            nc.gpsimd.collective_compute(
                                "AllGather", mybir.AluOpType.bypass,
                                replica_groups=groups,
                                ins=[t_own[c * CHK:(c + 1) * CHK, :]],
                                outs=[t_full1[c * QROWS:(c + 1) * QROWS, :]],
                            )

                pos = 0
                while pos < SHARD_PAD:
                    n = min(T, SHARD_PAD - pos)
                    xt = xf.tile([F_IN, T], fp32, tag="xt")
                    nc.sync.dma_start(xt[:, :n], xT[:, pos:pos + n])
                    h0p = xfp.tile([H, T], fp32, tag="h0p")
                    nc.tensor.matmul(h0p[:, :n], lhsT=w_in_t[:], rhs=xt[:, :n],
                                     start=True, stop=True)
                    h0s = xf.tile([H, T], fp32, tag="h0s")
                    nc.vector.tensor_scalar_add(h0s[:, :n], h0p[:, :n], bin_t[:, :1])
                    t1p = xfp.tile([H, T], fp32, tag="t1p")
                    nc.tensor.matmul(t1p[:, :n], lhsT=w1_t[:], rhs=h0s[:, :n],
                                     start=True, stop=True)
                    t1s = xf.tile([H, T], fp32, tag="t1s")
                    nc.scalar.activation(t1s[:, :n], t1p[:, :n],
                                         mybir.ActivationFunctionType.Relu,
                                         bias=b1_t[:, :1], scale=1.0)
                    # transpose to node-major in 128-col pieces, scale by dinv
                    for s in range(0, n, 128):
                        w = min(128, n - s)
                        bidx = (pos + s) // 128
                        tp = xfp.tile([128, H], fp32, tag="tp")
                        nc.tensor.transpose(tp[:w, :], t1s[:, s:s + w], ident[:H, :H])
                        tn = xf.tile([128, H], fp32, tag="tn")
                        nc.scalar.activation(tn[:w, :], tp[:w, :],
                                             mybir.ActivationFunctionType.Copy,
                                             bias=0.0, scale=dinv_t[:w, bidx:bidx + 1])
                        nc.sync.dma_start(t_own[pos + s:pos + s + w, :], tn[:w, :])
                    pos += n
                    emit_ag1(pos)

            # ---- AllGather t1: flush chunks not yet emitted ----
            if no_ag:
                nc.sync.dma_start(t_full1[0:SHARD_PAD, :], t_own[:])
            else:
                emit_ag1(SHARD_PAD + 512)

            # ---- propagate + fused next transform, per block ----
            def propagate_layer(t_full, emit_epilogue, after_group=None):
                with tc.tile_pool(name="pg", bufs=3) as pg, \
                     tc.tile_pool(name="pgp", bufs=1, space="PSUM") as pgp, \
                     tc.tile_pool(name="gbp", bufs=1) as gbp, \
                     tc.tile_pool(name="ohp", bufs=3) as ohp:
                    # persistent gather buffers, zeroed once
                    gatb = []
                    for i in range(4):
                        g = gbp.tile([128, JGMAX, H], fp32, name=f"gatb{i}")
                        nc.vector.memset(g[:], 0.0)
                        gatb.append(g)
                    call_i = 0
                    for b0 in range(0, B, GMERGE):
                        blocks = list(range(b0, min(b0 + GMERGE, B)))
                        jall = sum(jqs[b][q] for b in blocks for q in range(NQ))
                        og = joff[blocks[0]][0]
                        idx_t = pg.tile([128, 8 * jall], i16, tag="idx",
                                        padded_shape=[128, 8 * (JGMAX + 1) * NQ])
                        nc.sync.dma_start(idx_t[:], colidx[:, 8 * og:8 * (og + jall)])
                        lr_t = pg.tile([128, jall], fp32, tag="lr",
                                       padded_shape=[128, (JGMAX + 1) * NQ])
                        nc.sync.dma_start(lr_t[:], lrow[:, og:og + jall])
                        accs = {}
                        for b in blocks:
                            a = pgp.tile([128, H], fp32, name=f"acc{b - b0}",
                                         tag=f"acc{b - b0}")
                            accs[b] = a
                        for q in range(NQ):
                            jg = sum(jqs[b][q] for b in blocks)
                            lo = joff[blocks[0]][q] - og
                            gat = gatb[call_i % 4]
                            call_i += 1
                            src = t_full[q * QROWS:(q + 1) * QROWS, :]
                            nc.gpsimd.dma_gather(
                                gat[:, :jg, :], src,
                                idx_t[:, 8 * lo:8 * (lo + jg)],
                                jg * 128, jg * 128, H,
                                elem_step=H, single_packet=False,
                            )
                            oh = ohp.tile([128, jg, 128], fp32, tag="oh",
                                          padded_shape=[128, JGMAX, 128])
                            nc.vector.tensor_tensor(
                                out=oh[:],
                                in0=iota_t[:, None, :].to_broadcast([128, jg, 128]),
                                in1=lr_t[:, lo:lo + jg, None].to_broadcast(
                                    [128, jg, 128]),
                                op=mybir.AluOpType.is_equal,
                            )
                            jrel = 0
                            for b in blocks:
                                for j in range(jrel, jrel + jqs[b][q]):
                                    nc.tensor.matmul(
                                        accs[b][:], lhsT=oh[:, j, :],
                                        rhs=gat[:, j, :],
                                        start=(q == 0 and j == jrel),
                                        stop=(q == NQ - 1
                                              and j == jrel + jqs[b][q] - 1),
                                    )
                                jrel += jqs[b][q]
                        for b in blocks:
                            emit_epilogue(b, accs[b], pg, pgp)
                        if after_group is not None:
                            after_group(blocks[-1] + 1)

            def epilogue1(b, acc, pg, pgp):
                # h1 = acc * dinv_dest ; t2 = relu(h1@W2 + b2) * dinv -> t_own2
                h1s = pg.tile([128, H], fp32, tag="h1s")
                nc.scalar.activation(h1s[:], acc[:],
                                     mybir.ActivationFunctionType.Copy,
                                     bias=0.0, scale=dinv_t[:, b:b + 1])
                h1tp = pgp.tile([H, 128], fp32, tag="h1tp")
                nc.tensor.transpose(h1tp[:], h1s[:], ident[:])
                h1ts = pg.tile([H, 128], fp32, tag="h1ts")
                nc.scalar.copy(h1ts[:], h1tp[:])
                t2p = pgp.tile([H, 128], fp32, tag="t2p")
                nc.tensor.matmul(t2p[:], lhsT=w2_t[:], rhs=h1ts[:],
                                 start=True, stop=True)
                t2ts = pg.tile([H, 128], fp32, tag="t2ts")
                nc.scalar.activation(t2ts[:], t2p[:],
                                     mybir.ActivationFunctionType.Relu,
                                     bias=b2_t[:, :1], scale=1.0)
                t2np = pgp.tile([128, H], fp32, tag="t2np")
                nc.tensor.transpose(t2np[:], t2ts[:], ident[:H, :H])
                t2n = pg.tile([128, H], fp32, tag="t2n")
                nc.scalar.activation(t2n[:], t2np[:],
                                     mybir.ActivationFunctionType.Copy,
                                     bias=0.0, scale=dinv_t[:, b:b + 1])
                nc.sync.dma_start(t_own2[b * 128:(b + 1) * 128, :], t2n[:])

            def epilogue2(b, acc, pg, pgp):
                # h2 = acc * dinv_dest ; outT block = W_out.T @ h2.T + b_out
                h2s = pg.tile([128, H], fp32, tag="h2s")
                nc.scalar.activation(h2s[:], acc[:],
                                     mybir.ActivationFunctionType.Copy,
                                     bias=0.0, scale=dinv_t[:, b:b + 1])
                h2tp = pgp.tile([H, 128], fp32, tag="h2tp")
                nc.tensor.transpose(h2tp[:], h2s[:], ident[:])
                h2ts = pg.tile([H, 128], fp32, tag="h2ts")
                nc.scalar.copy(h2ts[:], h2tp[:])
                op = pgp.tile([H, 128], fp32, tag="op")
                nc.tensor.matmul(op[:], lhsT=w_out_t[:], rhs=h2ts[:],
                                 start=True, stop=True)
                os_ = pg.tile([H, 128], fp32, tag="os")
                nc.vector.tensor_scalar_add(os_[:], op[:], bout_t[:, :1])
                nc.sync.dma_start(outT[:, b * 128:(b + 1) * 128], os_[:])

            if phases >= 2:
                ag2_done = [False] * NQ

                def emit_ag2(blocks_done):
                    if no_ag:
                        return
                    for c in range(NQ):
                        if (not ag2_done[c]
                                and blocks_done * 128 >= (c + 1) * CHK + 512):
                            ag2_done[c] = True
                            nc.gpsimd.collective_compute(
                                "AllGather", mybir.AluOpType.bypass,
                                replica_groups=groups,
                                ins=[t_own2[c * CHK:(c + 1) * CHK, :]],
                                outs=[t_full2[c * QROWS:(c + 1) * QROWS, :]],
                            )

                propagate_layer(t_full1, epilogue1, after_group=emit_ag2)
                if no_ag:
                    nc.sync.dma_start(t_full2[0:SHARD_PAD, :], t_own2[:])
                else:
                    emit_ag2(B + 4)
            if phases >= 3:
                propagate_layer(t_full2, epilogue2)
            if phases < 3:
                # outT still must be written (ExternalOutput): dump t_own
                z = cpool.tile([H, 128], fp32)
                nc.vector.memset(z[:], 0.0)
                for b in range(B):
                    nc.sync.dma_start(outT[:, b * 128:(b + 1) * 128], z[:])

    nc.compile()
    return nc


def kernel(**inputs):
    global LAST_RESULTS
    from concourse.bass_utils import run_bass_kernel_spmd

    x = np.asarray(inputs["x"], dtype=np.float32)
    edge_index = np.asarray(inputs["edge_index"])

    colidx, lrowv, dinv, jq_tab = _preprocess_edges(edge_index)

    key = jq_tab.tobytes()
    if key not in _BUILD_CACHE:
        _BUILD_CACHE[key] = _build(jq_tab)
    nc = _BUILD_CACHE[key]

    dinv_pad = np.zeros(NPAD, np.float32)
    iota = np.ascontiguousarray(
        np.broadcast_to(np.arange(128, dtype=np.float32), (128, 128)))

    in_maps = []
    for k in range(NCORES):
        lo, hi = k * SHARD, (k + 1) * SHARD
        xT_k = np.zeros((F_IN, SHARD_PAD), np.float32)
        xT_k[:, :SHARD] = x[lo:hi].T
        dv = np.zeros(SHARD_PAD, np.float32)
        dv[:SHARD] = dinv[lo:hi]
        dinv_cols = np.ascontiguousarray(dv.reshape(B, 128).T)
        in_maps.append({
            "xT": xT_k,
            "W_in": np.asarray(inputs["W_in"], np.float32),
            "W1": np.asarray(inputs["W1"], np.float32),
            "W2": np.asarray(inputs["W2"], np.float32),
            "W_out": np.asarray(inputs["W_out"], np.float32),
            "b_in": np.asarray(inputs["b_in"], np.float32).reshape(H, 1),
            "b1": np.asarray(inputs["b1"], np.float32).reshape(H, 1),
            "b2": np.asarray(inputs["b2"], np.float32).reshape(H, 1),
            "b_out": np.asarray(inputs["b_out"], np.float32).reshape(H, 1),
            "dinv_cols": dinv_cols,
            "iota_in": iota,
            "colidx": colidx[k],
            "lrow": lrowv[k],
        })

    trace = bool(int(os.environ.get("GCN_TRACE", "0")))
    res = run_bass_kernel_spmd(nc, in_maps, core_ids=list(range(NCORES)),
                               trace=trace)
    LAST_RESULTS = res

    out = np.empty((N, H), np.float32)
    for k in range(NCORES):
        out[k * SHARD:(k + 1) * SHARD] = res.results[k]["outT"].T[:SHARD]
    return out

